# revision 4
# baseline (speedup 1.0000x reference)
"""Trainium2 Bass kernel for the NSDE model (Euler-Maruyama scan + MLPs).

Strategy:
  - Data-parallel over batch: 16384 rows -> 8 cores x 2048 rows.
  - Only the 20 time slices of x_path that the scan actually reads are
    shipped to the device (indices computed on host from t_span).
  - Feature-major layout on chip: activations are [feature, batch] so every
    matmul uses the weight matrix directly as lhsT (out = W^T @ actT), and
    biases are per-partition scalars.
  - 64-feature tensors (h, x, zs, g1, sigmoid, drift-out) are "packed":
    partitions 0-63 hold features of batch half A (rows 0..1023), partitions
    64-127 hold features of batch half B (rows 1024..2047). Matmuls touching
    the packed halves use PE row/col tiling (tile_position) with duplicated
    weight copies on partitions 64-127, so both halves run concurrently on
    disjoint quadrants of the PE array.
  - The h carry update is accumulated in PSUM: psum3 = dt*(z2 @ dW3) + h
    (h added via an identity-matrix matmul), then a single fused
    scalar_tensor_tensor op computes h' = (psum3 + dt*db3) + sigmoid*zs.
  - dt and sqrt(dt) folds are done on host: dW3*dt per step, db3*dt per
    step, and the noise is pre-scaled zs = dW * gscale * sqrt(dt).
"""

import os
from contextlib import ExitStack

import numpy as np

import concourse.bass as bass
import concourse.mybir as mybir
import concourse.tile as tile
from concourse import bacc
from concourse.bass_utils import run_bass_kernel_spmd

F32 = mybir.dt.float32
AF = mybir.ActivationFunctionType
ALU = mybir.AluOpType

STEPS = 20
NCORES = 8
B = 16384
BC = B // NCORES  # per-core batch: 2048
HALF = BC // 2  # packed half: 1024
H = 64  # hidden state size
FX = 64  # x feature size
DW = 128  # drift MLP width

_CACHE = {}


def _build():
    if "nc" in _CACHE:
        return _CACHE["nc"]
    nc = bacc.Bacc("TRN2", target_bir_lowering=False, debug=False)

    def din(name, shape):
        return nc.dram_tensor(name, shape, F32, kind="ExternalInput")

    d_xt = din("xt", [STEPS, 128, HALF])  # packed x slices (feature-major)
    d_zst = din("zst", [STEPS, 128, HALF])  # packed gscale*sqrt(dt)*dW
    d_w1h = din("w1h", [128, DW])  # dW1[:64] duplicated on partitions 64-127
    d_w1x = din("w1x", [128, DW])  # dW1[64:] duplicated
    d_w2 = din("w2", [DW, DW])
    d_w3s = din("w3s", [STEPS, DW, H])  # dW3 * dt_k
    d_gw1 = din("gw1", [128, H])  # gW1 duplicated
    d_gw2 = din("gw2", [128, H])  # gW2 duplicated
    d_id = din("ident", [128, H])  # I64 duplicated
    d_b1 = din("b1", [DW, 1])
    d_b2 = din("b2", [DW, 1])
    d_dtb3 = din("dtb3", [128, STEPS])  # dt_k * db3, packed-duplicated
    d_gb1 = din("gb1", [128, 1])  # gb1 duplicated
    d_gb2 = din("gb2", [128, 1])  # gb2 duplicated
    d_rw1 = din("rw1", [H, 32])
    d_rb1 = din("rb1", [32, 1])
    d_rw2 = din("rw2", [32, 2])
    d_rb2 = din("rb2", [2, 1])
    d_out = nc.dram_tensor("out", [2, BC], F32, kind="ExternalOutput")

    with ExitStack() as ctx:
        tc = ctx.enter_context(tile.TileContext(nc))
        consts = ctx.enter_context(tc.tile_pool(name="consts", bufs=1))
        xzp = ctx.enter_context(tc.tile_pool(name="xzp", bufs=3))
        hp = ctx.enter_context(tc.tile_pool(name="hp", bufs=2))
        wk = ctx.enter_context(tc.tile_pool(name="wk", bufs=2))
        pp = ctx.enter_context(tc.tile_pool(name="pp", bufs=4, space="PSUM"))

        def cload(dram_ap, shape, name):
            t = consts.tile(shape, F32, name=name, tag=name)
            nc.sync.dma_start(t[:], dram_ap)
            return t

        w1h = cload(d_w1h[:, :], [128, DW], "w1h")
        w1x = cload(d_w1x[:, :], [128, DW], "w1x")
        w2 = cload(d_w2[:, :], [DW, DW], "w2")
        w3s = cload(d_w3s[:, :, :].rearrange("k p m -> p k m"), [DW, STEPS, H], "w3s")
        gw1 = cload(d_gw1[:, :], [128, H], "gw1")
        gw2 = cload(d_gw2[:, :], [128, H], "gw2")
        idn = cload(d_id[:, :], [128, H], "idn")
        b1 = cload(d_b1[:, :], [DW, 1], "b1")
        b2 = cload(d_b2[:, :], [DW, 1], "b2")
        dtb3 = cload(d_dtb3[:, :], [128, STEPS], "dtb3")
        gb1 = cload(d_gb1[:, :], [128, 1], "gb1")
        gb2 = cload(d_gb2[:, :], [128, 1], "gb2")
        rw1 = cload(d_rw1[:, :], [H, 32], "rw1")
        rb1 = cload(d_rb1[:, :], [32, 1], "rb1")
        rw2 = cload(d_rw2[:, :], [32, 2], "rw2")
        rb2 = cload(d_rb2[:, :], [2, 1], "rb2")

        h_cur = hp.tile([128, HALF], F32, name="h", tag="h")
        nc.vector.memset(h_cur[:], 0.0)

        for k in range(STEPS):
            xk = xzp.tile([128, HALF], F32, name="xk", tag="xk")
            nc.sync.dma_start(xk[:], d_xt[k])
            zk = xzp.tile([128, HALF], F32, name="zk", tag="zk")
            nc.sync.dma_start(zk[:], d_zst[k])

            # ---- diffusion branch (packed, depends only on h) ----
            psg = pp.tile([128, HALF], F32, name="psg", tag="pp")
            for j in range(2):
                sl = slice(j * 512, (j + 1) * 512)
                nc.tensor.matmul(
                    psg[0:64, sl], gw1[0:64, :], h_cur[0:64, sl],
                    start=True, stop=True,
                )
                nc.tensor.matmul(
                    psg[64:128, sl], gw1[64:128, :], h_cur[64:128, sl],
                    start=True, stop=True, tile_position=(64, 64),
                )
            g1 = wk.tile([128, HALF], F32, name="g1", tag="g1")
            nc.scalar.activation(g1[:], psg[:], AF.Relu, bias=gb1[:])

            pss = pp.tile([128, HALF], F32, name="pss", tag="pp")
            for j in range(2):
                sl = slice(j * 512, (j + 1) * 512)
                nc.tensor.matmul(
                    pss[0:64, sl], gw2[0:64, :], g1[0:64, sl],
                    start=True, stop=True,
                )
                nc.tensor.matmul(
                    pss[64:128, sl], gw2[64:128, :], g1[64:128, sl],
                    start=True, stop=True, tile_position=(64, 64),
                )
            sg = wk.tile([128, HALF], F32, name="sg", tag="sg")
            nc.scalar.activation(sg[:], pss[:], AF.Sigmoid, bias=gb2[:])

            # noise term t = sigmoid * zs on GPSIMD (SBUF-only op)
            tt = wk.tile([128, HALF], F32, name="tt", tag="tt")
            nc.gpsimd.tensor_mul(tt[:], sg[:], zk[:])

            # ---- drift layer 1: z1 = relu(W1h^T h + W1x^T x + b1) ----
            # z1 is [128 feats, 2048 batch]; half A from packed rows 0-63,
            # half B from packed rows 64-127 (concurrent PE row groups).
            ps1a = pp.tile([128, HALF], F32, name="ps1a", tag="pp")
            ps1b = pp.tile([128, HALF], F32, name="ps1b", tag="pp")
            for j in range(2):
                sl = slice(j * 512, (j + 1) * 512)
                nc.tensor.matmul(
                    ps1a[:, sl], w1h[0:64, :], h_cur[0:64, sl],
                    start=True, stop=False,
                )
                nc.tensor.matmul(
                    ps1a[:, sl], w1x[0:64, :], xk[0:64, sl],
                    start=False, stop=True,
                )
                nc.tensor.matmul(
                    ps1b[:, sl], w1h[64:128, :], h_cur[64:128, sl],
                    start=True, stop=False, tile_position=(64, 0),
                )
                nc.tensor.matmul(
                    ps1b[:, sl], w1x[64:128, :], xk[64:128, sl],
                    start=False, stop=True, tile_position=(64, 0),
                )
            z1 = wk.tile([128, 2 * HALF], F32, name="z1", tag="z1")
            nc.scalar.activation(z1[:, 0:HALF], ps1a[:], AF.Relu, bias=b1[:])
            nc.vector.tensor_scalar(
                z1[:, HALF:], ps1b[:], b1[:], 0.0, ALU.add, ALU.max
            )

            # ---- drift layer 2: z2 = relu(W2^T z1 + b2) ----
            ps2a = pp.tile([128, HALF], F32, name="ps2a", tag="pp")
            ps2b = pp.tile([128, HALF], F32, name="ps2b", tag="pp")
            for j in range(2):
                sl = slice(j * 512, (j + 1) * 512)
                sl2 = slice(HALF + j * 512, HALF + (j + 1) * 512)
                nc.tensor.matmul(
                    ps2a[:, sl], w2[:, :], z1[:, sl], start=True, stop=True
                )
                nc.tensor.matmul(
                    ps2b[:, sl], w2[:, :], z1[:, sl2], start=True, stop=True
                )
            z2 = wk.tile([128, 2 * HALF], F32, name="z2", tag="z2")
            nc.vector.tensor_scalar(
                z2[:, 0:HALF], ps2a[:], b2[:], 0.0, ALU.add, ALU.max
            )
            nc.vector.tensor_scalar(
                z2[:, HALF:], ps2b[:], b2[:], 0.0, ALU.add, ALU.max
            )

            # ---- drift out + h carry in PSUM: ps3 = dt*(z2 @ dW3) + h ----
            ps3 = pp.tile([128, HALF], F32, name="ps3", tag="pp")
            for j in range(2):
                sl = slice(j * 512, (j + 1) * 512)
                sl2 = slice(HALF + j * 512, HALF + (j + 1) * 512)
                # half A -> ps3[0:64]
                nc.tensor.matmul(
                    ps3[0:64, sl], w3s[:, k, :], z2[:, sl],
                    start=True, stop=False,
                )
                nc.tensor.matmul(
                    ps3[0:64, sl], idn[0:64, :], h_cur[0:64, sl],
                    start=False, stop=True,
                )
                # half B -> ps3[64:128] (col group 2-3)
                nc.tensor.matmul(
                    ps3[64:128, sl], w3s[:, k, :], z2[:, sl2],
                    start=True, stop=False, tile_position=(0, 64),
                )
                nc.tensor.matmul(
                    ps3[64:128, sl], idn[64:128, :], h_cur[64:128, sl],
                    start=False, stop=True, tile_position=(64, 64),
                )
            # h' = (ps3 + dt*db3) + sigmoid*zs
            h_new = hp.tile([128, HALF], F32, name="h", tag="h")
            nc.vector.scalar_tensor_tensor(
                h_new[:], ps3[:], dtb3[:, k : k + 1], tt[:], ALU.add, ALU.add
            )
            h_cur = h_new

        # ---- readout: out = relu(h @ rW1 + rb1) @ rW2 + rb2 ----
        h_unp = wk.tile([H, BC], F32, name="h_unp", tag="h_unp")
        nc.sync.dma_start(h_unp[:, 0:HALF], h_cur[0:64, :])
        nc.sync.dma_start(h_unp[:, HALF:], h_cur[64:128, :])

        r1 = wk.tile([32, BC], F32, name="r1", tag="r1")
        for half in range(2):
            psr = pp.tile([128, HALF], F32, name="psr", tag="pp")
            for j in range(2):
                sl_in = slice(half * HALF + j * 512, half * HALF + (j + 1) * 512)
                sl_ps = slice(j * 512, (j + 1) * 512)
                nc.tensor.matmul(
                    psr[0:32, sl_ps], rw1[:, :], h_unp[:, sl_in],
                    start=True, stop=True,
                )
            nc.scalar.activation(
                r1[:, half * HALF : (half + 1) * HALF],
                psr[0:32, :],
                AF.Relu,
                bias=rb1[:],
            )

        osb = wk.tile([2, BC], F32, name="osb", tag="osb")
        for half in range(2):
            pso = pp.tile([128, HALF], F32, name="pso", tag="pp")
            for j in range(2):
                sl_in = slice(half * HALF + j * 512, half * HALF + (j + 1) * 512)
                sl_ps = slice(j * 512, (j + 1) * 512)
                nc.tensor.matmul(
                    pso[0:2, sl_ps], rw2[:, :], r1[:, sl_in],
                    start=True, stop=True,
                )
            nc.scalar.activation(
                osb[:, half * HALF : (half + 1) * HALF],
                pso[0:2, :],
                AF.Identity,
                bias=rb2[:],
            )
        nc.sync.dma_start(d_out[:, :], osb[:])

    nc.compile()
    _CACHE["nc"] = nc
    return nc


def _dup(a):
    return np.ascontiguousarray(np.concatenate([a, a], axis=0), dtype=np.float32)


def _prep_in_maps(inputs):
    xp = np.asarray(inputs["x_path"], dtype=np.float32)
    t_span = np.asarray(inputs["t_span"], dtype=np.float32)
    dw = np.asarray(inputs["dW"], dtype=np.float32)

    Tm1 = np.int32(xp.shape[1] - 1)
    t_max = t_span[-1]
    idx = np.clip(
        (t_span[:-1] / t_max * np.float32(Tm1)).astype(np.int32), 0, Tm1
    )
    dts = (t_span[1:] - t_span[:-1]).astype(np.float32)
    sq = np.sqrt(dts).astype(np.float32)

    gscale = np.asarray(inputs["gscale"], dtype=np.float32)
    w1 = np.asarray(inputs["dW1"], dtype=np.float32)
    w2 = np.ascontiguousarray(np.asarray(inputs["dW2"], dtype=np.float32))
    w3 = np.asarray(inputs["dW3"], dtype=np.float32)
    db1 = np.asarray(inputs["db1"], dtype=np.float32)
    db2 = np.asarray(inputs["db2"], dtype=np.float32)
    db3 = np.asarray(inputs["db3"], dtype=np.float32)
    gw1 = np.asarray(inputs["gW1"], dtype=np.float32)
    gw2 = np.asarray(inputs["gW2"], dtype=np.float32)
    gb1 = np.asarray(inputs["gb1"], dtype=np.float32)
    gb2 = np.asarray(inputs["gb2"], dtype=np.float32)
    rw1 = np.ascontiguousarray(np.asarray(inputs["rW1"], dtype=np.float32))
    rb1 = np.asarray(inputs["rb1"], dtype=np.float32)
    rw2 = np.ascontiguousarray(np.asarray(inputs["rW2"], dtype=np.float32))
    rb2 = np.asarray(inputs["rb2"], dtype=np.float32)

    common = {
        "w1h": _dup(w1[:H]),
        "w1x": _dup(w1[H:]),
        "w2": w2,
        "w3s": np.ascontiguousarray(w3[None, :, :] * dts[:, None, None]),
        "gw1": _dup(gw1),
        "gw2": _dup(gw2),
        "ident": _dup(np.eye(H, dtype=np.float32)),
        "b1": np.ascontiguousarray(db1.reshape(DW, 1)),
        "b2": np.ascontiguousarray(db2.reshape(DW, 1)),
        "dtb3": _dup((dts[:, None] * db3[None, :]).T),  # [128, STEPS]
        "gb1": _dup(gb1.reshape(H, 1)),
        "gb2": _dup(gb2.reshape(H, 1)),
        "rw1": rw1,
        "rb1": np.ascontiguousarray(rb1.reshape(32, 1)),
        "rw2": rw2,
        "rb2": np.ascontiguousarray(rb2.reshape(2, 1)),
    }

    xg = xp[:, idx, :]  # [B, STEPS, F]
    zsc = gscale[None, :] * sq[:, None]  # [STEPS, F]

    in_maps = []
    for c in range(NCORES):
        rows = slice(c * BC, (c + 1) * BC)
        # x: (b2, b', k, f) -> (k, b2, f, b') -> [STEPS, 128, HALF]
        xt = np.ascontiguousarray(
            xg[rows]
            .reshape(2, HALF, STEPS, FX)
            .transpose(2, 0, 3, 1)
            .reshape(STEPS, 128, HALF)
        )
        zc = dw[:, rows, :] * zsc[:, None, :]  # [STEPS, BC, H]
        zst = np.ascontiguousarray(
            zc.reshape(STEPS, 2, HALF, H)
            .transpose(0, 1, 3, 2)
            .reshape(STEPS, 128, HALF)
        )
        m = dict(common)
        m["xt"] = xt
        m["zst"] = zst
        in_maps.append(m)
    return in_maps


def kernel(**inputs):
    nc = _build()
    in_maps = _prep_in_maps(inputs)
    run_kwargs = dict(_CACHE.get("run_kwargs", {}))
    res = run_bass_kernel_spmd(nc, in_maps, list(range(NCORES)), **run_kwargs)
    _CACHE["last_results"] = res
    mu = np.concatenate([res.results[c]["out"][0] for c in range(NCORES)])
    ls = np.concatenate([res.results[c]["out"][1] for c in range(NCORES)])
    return mu, ls


# revision 8
# speedup vs baseline: 1.5453x; 1.5453x over previous
"""Trainium2 Bass kernel for the NSDE model (Euler-Maruyama scan + MLPs).

Strategy:
  - Data-parallel over batch: 16384 rows -> 8 cores x 2048 rows.
  - Only the 20 time slices of x_path that the scan actually reads are
    shipped to the device (indices computed on host from t_span).
  - Feature-major layout on chip: activations are [feature, batch] so every
    matmul uses the weight matrix directly as lhsT (out = W^T @ actT), and
    biases are per-partition scalars.
  - 64-feature tensors (h, x, zs, g1, sigmoid, drift-out) are "packed":
    partitions 0-63 hold features of batch half A (rows 0..1023), partitions
    64-127 hold features of batch half B (rows 1024..2047). Matmuls touching
    the packed halves use PE row/col tiling (tile_position) with duplicated
    weight copies on partitions 64-127, so both halves run concurrently on
    disjoint quadrants of the PE array.
  - The h carry update is accumulated in PSUM: psum3 = dt*(z2 @ dW3) + h
    (h added via an identity-matrix matmul), then a single fused
    scalar_tensor_tensor op computes h' = (psum3 + dt*db3) + sigmoid*zs.
  - dt and sqrt(dt) folds are done on host: dW3*dt per step, db3*dt per
    step, and the noise is pre-scaled zs = dW * gscale * sqrt(dt).
  - Matmul operands are bitcast to float32r: same 4-byte layout but the PE
    streams 1 row/cycle (plain float32 lowers to two half-rate passes).
"""

import os
from contextlib import ExitStack

import numpy as np

import concourse.bass as bass
import concourse.mybir as mybir
import concourse.tile as tile
from concourse import bacc
from concourse.bass_utils import run_bass_kernel_spmd

F32 = mybir.dt.float32
F32R = mybir.dt.float32r
AF = mybir.ActivationFunctionType
ALU = mybir.AluOpType

# dtype used for matmul operands (bitcast view; no data movement)
MM_DT = F32R

STEPS = 20
NCORES = 8
B = 16384
BC = B // NCORES  # per-core batch: 2048
HALF = BC // 2  # packed half: 1024
H = 64  # hidden state size
FX = 64  # x feature size
DW = 128  # drift MLP width

_CACHE = {}


def _build():
    if "nc" in _CACHE:
        return _CACHE["nc"]

    def mm(ap):
        return ap

    nc = bacc.Bacc("TRN2", target_bir_lowering=False, debug=False)

    def din(name, shape, dt=F32):
        return nc.dram_tensor(name, shape, dt, kind="ExternalInput")

    d_xt = din("xt", [STEPS, 128, HALF], MM_DT)  # packed x slices (feature-major)
    d_zst = din("zst", [STEPS, 128, HALF])  # packed gscale*sqrt(dt)*dW
    d_w1h = din("w1h", [128, DW], MM_DT)  # dW1[:64] duplicated on partitions 64-127
    d_w1x = din("w1x", [128, DW], MM_DT)  # dW1[64:] duplicated
    d_w2 = din("w2", [DW, DW], MM_DT)
    d_w3lo = din("w3lo", [STEPS, DW, DW], MM_DT)  # [dW3*dt_k | 0]
    d_w3hi = din("w3hi", [STEPS, DW, DW], MM_DT)  # [0 | dW3*dt_k]
    d_gw1 = din("gw1", [128, DW], MM_DT)  # blockdiag(gW1, gW1)
    d_gw2 = din("gw2", [128, DW], MM_DT)  # blockdiag(gW2, gW2)
    d_id = din("ident", [128, DW], MM_DT)  # 128x128 identity
    d_b1 = din("b1", [DW, 1])
    d_b2 = din("b2", [DW, 1])
    d_dtb3 = din("dtb3", [128, STEPS])  # dt_k * db3, packed-duplicated
    d_gb1 = din("gb1", [128, 1])  # gb1 duplicated
    d_gb2 = din("gb2", [128, 1])  # gb2 duplicated
    d_rw1 = din("rw1", [H, 32], MM_DT)
    d_rb1 = din("rb1", [32, 1])
    d_rw2 = din("rw2", [32, 2], MM_DT)
    d_rb2 = din("rb2", [2, 1])
    d_h0 = din("h0", [128, HALF], MM_DT)  # zeros (f32r memset unsupported)
    d_out = nc.dram_tensor("out", [2, BC], F32, kind="ExternalOutput")

    with ExitStack() as ctx:
        tc = ctx.enter_context(tile.TileContext(nc))
        consts = ctx.enter_context(tc.tile_pool(name="consts", bufs=1))
        xzp = ctx.enter_context(tc.tile_pool(name="xzp", bufs=3))
        hp = ctx.enter_context(tc.tile_pool(name="hp", bufs=2))
        wk = ctx.enter_context(tc.tile_pool(name="wk", bufs=2))
        pp = ctx.enter_context(tc.tile_pool(name="pp", bufs=4, space="PSUM"))

        def cload(dram_ap, shape, name, dt=F32):
            t = consts.tile(shape, dt, name=name, tag=name)
            nc.sync.dma_start(t[:], dram_ap)
            return t

        w1h = cload(d_w1h[:, :], [128, DW], "w1h", MM_DT)
        w1x = cload(d_w1x[:, :], [128, DW], "w1x", MM_DT)
        w2 = cload(d_w2[:, :], [DW, DW], "w2", MM_DT)
        w3lo = cload(d_w3lo[:, :, :].rearrange("k p m -> p k m"), [DW, STEPS, DW], "w3lo", MM_DT)
        w3hi = cload(d_w3hi[:, :, :].rearrange("k p m -> p k m"), [DW, STEPS, DW], "w3hi", MM_DT)
        gw1 = cload(d_gw1[:, :], [128, DW], "gw1", MM_DT)
        gw2 = cload(d_gw2[:, :], [128, DW], "gw2", MM_DT)
        idn = cload(d_id[:, :], [128, DW], "idn", MM_DT)
        b1 = cload(d_b1[:, :], [DW, 1], "b1")
        b2 = cload(d_b2[:, :], [DW, 1], "b2")
        dtb3 = cload(d_dtb3[:, :], [128, STEPS], "dtb3")
        gb1 = cload(d_gb1[:, :], [128, 1], "gb1")
        gb2 = cload(d_gb2[:, :], [128, 1], "gb2")
        rw1 = cload(d_rw1[:, :], [H, 32], "rw1", MM_DT)
        rb1 = cload(d_rb1[:, :], [32, 1], "rb1")
        rw2 = cload(d_rw2[:, :], [32, 2], "rw2", MM_DT)
        rb2 = cload(d_rb2[:, :], [2, 1], "rb2")

        h_cur = hp.tile([128, HALF], MM_DT, name="h", tag="h")
        nc.sync.dma_start(h_cur[:], d_h0[:, :])

        for k in range(STEPS):
            xk = xzp.tile([128, HALF], MM_DT, name="xk", tag="xk")
            nc.sync.dma_start(xk[:], d_xt[k])
            zk = xzp.tile([128, HALF], F32, name="zk", tag="zk")
            nc.sync.dma_start(zk[:], d_zst[k])

            # ---- diffusion branch (packed, depends only on h) ----
            psg = pp.tile([128, HALF], F32, name="psg", tag="pp")
            for j in range(2):
                sl = slice(j * 512, (j + 1) * 512)
                nc.tensor.matmul(
                    psg[:, sl], mm(gw1[:, :]), mm(h_cur[:, sl]),
                    start=True, stop=True,
                )
            g1 = wk.tile([128, HALF], MM_DT, name="g1", tag="g1")
            nc.scalar.activation(g1[:], psg[:], AF.Relu, bias=gb1[:])

            pss = pp.tile([128, HALF], F32, name="pss", tag="pp")
            for j in range(2):
                sl = slice(j * 512, (j + 1) * 512)
                nc.tensor.matmul(
                    pss[:, sl], mm(gw2[:, :]), mm(g1[:, sl]),
                    start=True, stop=True,
                )
            sg = wk.tile([128, HALF], F32, name="sg", tag="sg")
            nc.scalar.activation(sg[:], pss[:], AF.Sigmoid, bias=gb2[:])

            # noise term t = sigmoid * zs on GPSIMD (SBUF-only op)
            tt = wk.tile([128, HALF], F32, name="tt", tag="tt")
            nc.gpsimd.tensor_mul(tt[:], sg[:], zk[:])

            # ---- drift layer 1: z1 = relu(W1h^T h + W1x^T x + b1) ----
            # z1 is [128 feats, 2048 batch]; half A from packed rows 0-63,
            # half B from packed rows 64-127 (concurrent PE row groups).
            ps1a = pp.tile([128, HALF], F32, name="ps1a", tag="pp")
            ps1b = pp.tile([128, HALF], F32, name="ps1b", tag="pp")
            for j in range(2):
                sl = slice(j * 512, (j + 1) * 512)
                nc.tensor.matmul(
                    ps1a[:, sl], mm(w1h[0:64, :]), mm(h_cur[0:64, sl]),
                    start=True, stop=False,
                )
                nc.tensor.matmul(
                    ps1a[:, sl], mm(w1x[0:64, :]), mm(xk[0:64, sl]),
                    start=False, stop=True,
                )
                nc.tensor.matmul(
                    ps1b[:, sl], mm(w1h[64:128, :]), mm(h_cur[64:128, sl]),
                    start=True, stop=False, tile_position=(64, 0),
                )
                nc.tensor.matmul(
                    ps1b[:, sl], mm(w1x[64:128, :]), mm(xk[64:128, sl]),
                    start=False, stop=True, tile_position=(64, 0),
                )
            z1 = wk.tile([128, 2 * HALF], MM_DT, name="z1", tag="z1")
            nc.scalar.activation(z1[:, 0:HALF], ps1a[:], AF.Relu, bias=b1[:])
            nc.vector.tensor_scalar(
                z1[:, HALF:], ps1b[:], b1[:], 0.0, ALU.add, ALU.max
            )

            # ---- drift layer 2: z2 = relu(W2^T z1 + b2) ----
            ps2a = pp.tile([128, HALF], F32, name="ps2a", tag="pp")
            ps2b = pp.tile([128, HALF], F32, name="ps2b", tag="pp")
            for j in range(2):
                sl = slice(j * 512, (j + 1) * 512)
                sl2 = slice(HALF + j * 512, HALF + (j + 1) * 512)
                nc.tensor.matmul(
                    ps2a[:, sl], mm(w2[:, :]), mm(z1[:, sl]), start=True, stop=True
                )
                nc.tensor.matmul(
                    ps2b[:, sl], mm(w2[:, :]), mm(z1[:, sl2]), start=True, stop=True
                )
            z2 = wk.tile([128, 2 * HALF], MM_DT, name="z2", tag="z2")
            nc.vector.tensor_scalar(
                z2[:, 0:HALF], ps2a[:], b2[:], 0.0, ALU.add, ALU.max
            )
            nc.vector.tensor_scalar(
                z2[:, HALF:], ps2b[:], b2[:], 0.0, ALU.add, ALU.max
            )

            # ---- drift out + h carry in PSUM: ps3 = dt*(z2 @ dW3) + h ----
            ps3 = pp.tile([128, HALF], F32, name="ps3", tag="pp")
            for j in range(2):
                sl = slice(j * 512, (j + 1) * 512)
                sl2 = slice(HALF + j * 512, HALF + (j + 1) * 512)
                # rows 0:64 get dt*dW3^T z2[halfA]; rows 64:128 the B half
                nc.tensor.matmul(
                    ps3[:, sl], mm(w3lo[:, k, :]), mm(z2[:, sl]),
                    start=True, stop=False,
                )
                nc.tensor.matmul(
                    ps3[:, sl], mm(w3hi[:, k, :]), mm(z2[:, sl2]),
                    start=False, stop=False,
                )
                nc.tensor.matmul(
                    ps3[:, sl], mm(idn[:, :]), mm(h_cur[:, sl]),
                    start=False, stop=True,
                )
            # h' = (ps3 + dt*db3) + sigmoid*zs
            h_new = hp.tile([128, HALF], MM_DT, name="h", tag="h")
            nc.vector.scalar_tensor_tensor(
                h_new[:], ps3[:], dtb3[:, k : k + 1], tt[:], ALU.add, ALU.add
            )
            h_cur = h_new

        # ---- readout: out = relu(h @ rW1 + rb1) @ rW2 + rb2 ----
        h_unp = wk.tile([H, BC], MM_DT, name="h_unp", tag="h_unp")
        nc.sync.dma_start(h_unp[:, 0:HALF], h_cur[0:64, :])
        nc.sync.dma_start(h_unp[:, HALF:], h_cur[64:128, :])

        r1 = wk.tile([32, BC], MM_DT, name="r1", tag="r1")
        for half in range(2):
            psr = pp.tile([128, HALF], F32, name="psr", tag="pp")
            for j in range(2):
                sl_in = slice(half * HALF + j * 512, half * HALF + (j + 1) * 512)
                sl_ps = slice(j * 512, (j + 1) * 512)
                nc.tensor.matmul(
                    psr[0:32, sl_ps], mm(rw1[:, :]), mm(h_unp[:, sl_in]),
                    start=True, stop=True,
                )
            nc.scalar.activation(
                r1[:, half * HALF : (half + 1) * HALF],
                psr[0:32, :],
                AF.Relu,
                bias=rb1[:],
            )

        osb = wk.tile([2, BC], F32, name="osb", tag="osb")
        for half in range(2):
            pso = pp.tile([128, HALF], F32, name="pso", tag="pp")
            for j in range(2):
                sl_in = slice(half * HALF + j * 512, half * HALF + (j + 1) * 512)
                sl_ps = slice(j * 512, (j + 1) * 512)
                nc.tensor.matmul(
                    pso[0:2, sl_ps], mm(rw2[:, :]), mm(r1[:, sl_in]),
                    start=True, stop=True,
                )
            nc.scalar.activation(
                osb[:, half * HALF : (half + 1) * HALF],
                pso[0:2, :],
                AF.Identity,
                bias=rb2[:],
            )
        nc.sync.dma_start(d_out[:, :], osb[:])

    nc.compile()
    _CACHE["nc"] = nc
    return nc


def _dup(a):
    return np.ascontiguousarray(np.concatenate([a, a], axis=0), dtype=np.float32)


def _blkdiag(a):
    n, m = a.shape
    out = np.zeros((2 * n, 2 * m), np.float32)
    out[:n, :m] = a
    out[n:, m:] = a
    return out


def _prep_in_maps(inputs):
    xp = np.asarray(inputs["x_path"], dtype=np.float32)
    t_span = np.asarray(inputs["t_span"], dtype=np.float32)
    dw = np.asarray(inputs["dW"], dtype=np.float32)

    Tm1 = np.int32(xp.shape[1] - 1)
    t_max = t_span[-1]
    idx = np.clip(
        (t_span[:-1] / t_max * np.float32(Tm1)).astype(np.int32), 0, Tm1
    )
    dts = (t_span[1:] - t_span[:-1]).astype(np.float32)
    sq = np.sqrt(dts).astype(np.float32)

    gscale = np.asarray(inputs["gscale"], dtype=np.float32)
    w1 = np.asarray(inputs["dW1"], dtype=np.float32)
    w2 = np.ascontiguousarray(np.asarray(inputs["dW2"], dtype=np.float32))
    w3 = np.asarray(inputs["dW3"], dtype=np.float32)
    db1 = np.asarray(inputs["db1"], dtype=np.float32)
    db2 = np.asarray(inputs["db2"], dtype=np.float32)
    db3 = np.asarray(inputs["db3"], dtype=np.float32)
    gw1 = np.asarray(inputs["gW1"], dtype=np.float32)
    gw2 = np.asarray(inputs["gW2"], dtype=np.float32)
    gb1 = np.asarray(inputs["gb1"], dtype=np.float32)
    gb2 = np.asarray(inputs["gb2"], dtype=np.float32)
    rw1 = np.ascontiguousarray(np.asarray(inputs["rW1"], dtype=np.float32))
    rb1 = np.asarray(inputs["rb1"], dtype=np.float32)
    rw2 = np.ascontiguousarray(np.asarray(inputs["rW2"], dtype=np.float32))
    rb2 = np.asarray(inputs["rb2"], dtype=np.float32)

    w3s = w3[None, :, :] * dts[:, None, None]  # [STEPS, DW, H]
    zpad = np.zeros_like(w3s)
    w3lo = np.ascontiguousarray(np.concatenate([w3s, zpad], axis=2))
    w3hi = np.ascontiguousarray(np.concatenate([zpad, w3s], axis=2))

    common = {
        "w1h": _dup(w1[:H]),
        "w1x": _dup(w1[H:]),
        "w2": w2,
        "w3lo": w3lo,
        "w3hi": w3hi,
        "gw1": _blkdiag(gw1),
        "gw2": _blkdiag(gw2),
        "ident": np.eye(DW, dtype=np.float32),
        "b1": np.ascontiguousarray(db1.reshape(DW, 1)),
        "b2": np.ascontiguousarray(db2.reshape(DW, 1)),
        "dtb3": _dup((dts[:, None] * db3[None, :]).T),  # [128, STEPS]
        "gb1": _dup(gb1.reshape(H, 1)),
        "gb2": _dup(gb2.reshape(H, 1)),
        "rw1": rw1,
        "rb1": np.ascontiguousarray(rb1.reshape(32, 1)),
        "rw2": rw2,
        "rb2": np.ascontiguousarray(rb2.reshape(2, 1)),
        "h0": np.zeros((128, HALF), np.float32),
    }

    xg = xp[:, idx, :]  # [B, STEPS, F]
    zsc = gscale[None, :] * sq[:, None]  # [STEPS, F]

    in_maps = []
    for c in range(NCORES):
        rows = slice(c * BC, (c + 1) * BC)
        # x: (b2, b', k, f) -> (k, b2, f, b') -> [STEPS, 128, HALF]
        xt = np.ascontiguousarray(
            xg[rows]
            .reshape(2, HALF, STEPS, FX)
            .transpose(2, 0, 3, 1)
            .reshape(STEPS, 128, HALF)
        )
        zc = dw[:, rows, :] * zsc[:, None, :]  # [STEPS, BC, H]
        zst = np.ascontiguousarray(
            zc.reshape(STEPS, 2, HALF, H)
            .transpose(0, 1, 3, 2)
            .reshape(STEPS, 128, HALF)
        )
        m = dict(common)
        m["xt"] = xt
        m["zst"] = zst
        in_maps.append(m)
    return in_maps


def kernel(**inputs):
    nc = _build()
    in_maps = _prep_in_maps(inputs)
    run_kwargs = dict(_CACHE.get("run_kwargs", {}))
    res = run_bass_kernel_spmd(nc, in_maps, list(range(NCORES)), **run_kwargs)
    _CACHE["last_results"] = res
    mu = np.concatenate([res.results[c]["out"][0] for c in range(NCORES)])
    ls = np.concatenate([res.results[c]["out"][1] for c in range(NCORES)])
    return mu, ls
